# revision 5
# baseline (speedup 1.0000x reference)
"""Trainium2 Bass kernel for masked attention-pooling (DmasifAttentionModule).

Reference computation (per sample b):
    proj   = x @ W.T + b                  # [N, D]
    scores = proj @ v                     # [N]
    scores = where(mask, scores, -1e9)
    w      = softmax(scores)              # [N]
    out    = w @ x                        # [D]

Optimizations (exact up to fp reassociation unless noted):
  1. scores = x @ (W.T @ v) + (b . v); softmax is shift-invariant, so the
     (b . v) constant drops out and the 34-GFLOP projection collapses to a
     matvec against u = v @ W (host-computed, 512 floats).
  2. Masked rows get softmax weight exactly 0, so only the ~50% valid rows
     participate. The host compacts each sample to its valid rows (padded
     to a common column count with zero rows) and streams only those.
  3. x and u ship as fp16: halves HBM traffic (the binding resource), runs
     the pooling matmul at full PE rate (fp32 = 4 passes) and keeps DVE
     elementwise ops in 2x_1p mode. Score accumulation stays fp32.
  4. The compacted shard is host-swizzled to partition-major
     [128, SPB, ncols, D] so the whole 4.45 MiB arrives as ONE dma_start
     with 17.4 KiB contiguous per partition (meas. 295 GB/s vs 245 for
     per-tile strided transfers). Double-buffered across For_i iterations.
  5. Scores: a DVE free-dim reduce only has a 1x uop (694 ns/[128,512]
     column) while plain tensor_tensor runs 2x (438 ns), so columns are
     split: ~half fused on DVE (scalar_tensor_tensor w/ accum), the rest
     as DVE 2x products + ScalarE Copy-with-accum reduce (872 ns, ScalarE
     is otherwise idle). Masking is a single posthoc [128,ncols]
     tensor_add of -3e8 per masked column (no fp16-range contortions).
  6. exp: one batched ScalarE activation per sample (bias = -C shift),
     fp16 out. No accum: Z is recovered on host from the e tensor itself
     (8.7 KiB DMA per core), so numerator and denominator use bit-identical
     weights.
  7. Pooling: TensorE matvec accumulation into PSUM [1,512] per sample
     (lhsT = e column [128,1] fp16, rhs = x column [128,512] fp16,
     216 ns each); ScalarE copies PSUM out, host divides by Z.

Per-core budgets at ncols=17 (8 cores, 2 samples each, data-parallel):
DMA ~15.1 us, DVE ~18.7 us, ACT ~19 us, PE ~8 us -> ~20 us/iter steady.
"""

import os
import sys

import numpy as np

for _p in ("/opt/trn_rl_repo", "/root/.axon_site/_ro/trn_rl_repo"):
    if os.path.isdir(_p) and _p not in sys.path:
        sys.path.append(_p)

import concourse.bacc as bacc
import concourse.tile as tile
from concourse import mybir
from concourse.bass_utils import run_bass_kernel_spmd

B, N, D = 16, 4096, 512
N_CORES = 8
SPB = B // N_CORES          # samples per core
C_SHIFT = 24.0              # constant exp-range shift (softmax-invariant)
MASKED_INIT = -3.0e8        # masked scores -> exp underflows to exactly 0
ACT_COLS = 10               # score columns per sample reduced on ScalarE

_F32 = mybir.dt.float32
_F16 = mybir.dt.float16
_CACHE = {}


def _act_col_set(ncols, k):
    """k columns of [0,ncols) routed to the ScalarE reduce, spread out."""
    order = list(range(0, ncols, 2)) + list(range(1, ncols, 2))
    return frozenset(order[:k])


def _build_program(ncols, loop_n=None, act_cols=None, mask_in_stt=None):
    """Program for samples compacted to `ncols` columns of 128 rows each.

    loop_n wraps the computation in a HW For_i loop (timing only).
    mask_in_stt is accepted for test.py compatibility and ignored."""
    if act_cols is None:
        act_cols = ACT_COLS
    acols = _act_col_set(ncols, min(act_cols, ncols))

    nc = bacc.Bacc("TRN2", target_bir_lowering=False, debug=False)
    x = nc.dram_tensor("x", [128, SPB, ncols, D], _F16,
                       kind="ExternalInput").ap()
    mb = nc.dram_tensor("mb", [SPB, 128, ncols], _F32,
                        kind="ExternalInput").ap()
    u = nc.dram_tensor("u", [128, D], _F16, kind="ExternalInput").ap()
    out = nc.dram_tensor("out", [SPB, D], _F32, kind="ExternalOutput").ap()
    eout = nc.dram_tensor("eout", [128, SPB, ncols], _F16,
                          kind="ExternalOutput").ap()

    with tile.TileContext(nc) as tc:
        with (
            tc.tile_pool(name="xp", bufs=2) as xp,
            tc.tile_pool(name="singles", bufs=1) as sg,
            tc.tile_pool(name="prod", bufs=8) as prp,
            tc.tile_pool(name="scratch", bufs=2) as scr,
            tc.tile_pool(name="smalls", bufs=2) as sm,
            tc.tile_pool(name="ps", bufs=2, space="PSUM") as psp,
        ):
            ones_sb = sg.tile([128, 1], _F32)
            nc.vector.memset(ones_sb[:], 1.0)
            shift_sb = sg.tile([128, 1], _F32)
            nc.vector.memset(shift_sb[:], -C_SHIFT)
            warm = sg.tile([128, 1], _F32)
            # Pull the exp table-set load (~2.7us) to t=0, under the DMAs.
            nc.scalar.activation(warm[:], ones_sb[:],
                                 mybir.ActivationFunctionType.Exp)

            u_sb = sg.tile([128, D], _F16)
            nc.sync.dma_start(out=u_sb[:], in_=u[:])
            mb_sb = sg.tile([128, SPB, ncols], _F32)
            nc.sync.dma_start(out=mb_sb[:], in_=mb.rearrange("s p c -> p s c"))

            s_sb = sg.tile([128, SPB, ncols], _F32)
            e_sb = sg.tile([128, SPB, ncols], _F16)
            ctx = (nc, xp, prp, scr, sm, psp, x, out, eout, u_sb, mb_sb,
                   shift_sb, s_sb, e_sb, ncols, acols)

            if loop_n is not None:
                with tc.For_i(0, loop_n, 1) as _i:
                    _emit_iteration(*ctx)
            else:
                _emit_iteration(*ctx)

    nc.compile()
    return nc


def _emit_iteration(nc, xp, prp, scr, sm, psp, x, out, eout, u_sb, mb_sb,
                    shift_sb, s_sb, e_sb, ncols, acols):
    # One DMA for the whole shard; double-buffered across iterations so the
    # transfer of iteration i+1 overlaps the compute of iteration i.
    xt = xp.tile([128, SPB, ncols, D], _F16, name="xt")
    nc.sync.dma_start(out=xt[:], in_=x[:])

    pool_ps = {s: psp.tile([1, D], _F32, name=f"pool_ps_{s}")
               for s in range(SPB)}

    for s in range(SPB):
        # Scores. DVE 1x fused column: s[c] = sum_d x*u (694 ns). ACT-routed
        # column: DVE 2x product tile (438 ns) + ScalarE Copy-accum reduce
        # (872 ns). The split keeps both engines ~equally loaded.
        for c in range(ncols):
            if c in acols:
                prod = prp.tile([128, D], _F16, name="prod")
                nc.vector.tensor_tensor(
                    out=prod[:], in0=xt[:, s, c, :], in1=u_sb[:],
                    op=mybir.AluOpType.mult)
                dump32 = scr.tile([128, D], _F32, name="dump32")
                nc.scalar.activation(
                    dump32[:], prod[:],
                    mybir.ActivationFunctionType.Copy,
                    accum_out=s_sb[:, s, c:c + 1])
            else:
                dump = scr.tile([128, D], _F16, name="dump")
                nc.vector.scalar_tensor_tensor(
                    out=dump[:], in0=xt[:, s, c, :],
                    scalar=0.0, in1=u_sb[:],
                    op0=mybir.AluOpType.add, op1=mybir.AluOpType.mult,
                    accum_out=s_sb[:, s, c:c + 1])
        # Masking: valid rows +0, padding rows -3e8 -> exp == 0 exactly.
        nc.vector.tensor_add(s_sb[:, s, :], s_sb[:, s, :], mb_sb[:, s, :])
        # e = exp(s - C), one batched op per sample, fp16 out for the PE.
        nc.scalar.activation(e_sb[:, s, :], s_sb[:, s, :],
                             mybir.ActivationFunctionType.Exp,
                             bias=shift_sb[:])
        # Pooling: accumulate e_c . x_c into PSUM [1, D].
        for c in range(ncols):
            nc.tensor.matmul(
                pool_ps[s][:],
                e_sb[:, s, c:c + 1],
                xt[:, s, c, :],
                start=(c == 0),
                stop=(c == ncols - 1),
            )
        # Ship the raw PSUM accumulator; host does out = raw/Z with Z from e.
        o_sb = sm.tile([1, D], _F32, name=f"o_{s}")
        nc.scalar.activation(o_sb[:], pool_ps[s][:],
                             mybir.ActivationFunctionType.Copy)
        nc.sync.dma_start(out=out[s:s + 1, :], in_=o_sb[:])
    nc.sync.dma_start(out=eout[:], in_=e_sb[:])


def _get_program(ncols):
    if ncols not in _CACHE:
        _CACHE[ncols] = _build_program(ncols)
    return _CACHE[ncols]


def _prep_inputs(x, flat_mask, W, v):
    """Compact to valid rows, swizzle partition-major; (in_maps, meta)."""
    x = np.ascontiguousarray(x, dtype=np.float32)
    flat_mask = np.asarray(flat_mask)
    W = np.asarray(W, dtype=np.float32)
    v = np.asarray(v, dtype=np.float32)
    # scores = x @ u + (b . v); the constant is dropped by softmax invariance.
    u = (v @ W).astype(np.float16)
    u_rep = np.ascontiguousarray(np.broadcast_to(u, (128, D)), dtype=np.float16)

    idxs = [np.nonzero(flat_mask[b] == 1)[0] for b in range(B)]
    counts = np.array([len(ix) for ix in idxs])
    ncols = max(1, int(-(-counts.max() // 128)))
    ncap = ncols * 128

    xc = np.zeros((B, ncap, D), dtype=np.float16)
    mbc = np.full((B, ncap), np.float32(MASKED_INIT), dtype=np.float32)
    for b in range(B):
        cnt = counts[b]
        if cnt:
            xc[b, :cnt] = x[b, idxs[b]]
            mbc[b, :cnt] = 0.0
    # row = col*128 + p  ->  [B, 128, ncols(, D)] partition-major
    xc = xc.reshape(B, ncols, 128, D).transpose(0, 2, 1, 3)
    mbc = np.ascontiguousarray(mbc.reshape(B, ncols, 128).transpose(0, 2, 1))

    in_maps = []
    for core in range(N_CORES):
        lo = core * SPB
        in_maps.append({
            # [128, SPB, ncols, D]
            "x": np.ascontiguousarray(xc[lo:lo + SPB].transpose(1, 0, 2, 3)),
            "mb": np.ascontiguousarray(mbc[lo:lo + SPB]),
            "u": u_rep,
        })
    meta = {"ncols": ncols, "mask_in_stt": False, "counts": counts}
    return in_maps, meta


def kernel(x, flat_mask, W, b, v, **_unused):
    in_maps, meta = _prep_inputs(x, flat_mask, W, v)
    nc = _get_program(meta["ncols"])
    res = run_bass_kernel_spmd(nc, in_maps, core_ids=list(range(N_CORES)))
    raw = np.concatenate([res.results[i]["out"] for i in range(N_CORES)],
                         axis=0)
    z = np.concatenate(
        [res.results[i]["eout"].astype(np.float32).sum(axis=(0, 2))
         for i in range(N_CORES)], axis=0)
    out = (raw / z[:, None]).astype(np.float32)
    if (meta["counts"] == 0).any():
        # Reference semantics for an all-masked sample: uniform mean pool.
        x = np.asarray(x, dtype=np.float32)
        for bi in np.nonzero(meta["counts"] == 0)[0]:
            out[bi] = x[bi].mean(axis=0)
    return out


# revision 8
# speedup vs baseline: 1.6555x; 1.6555x over previous
"""Trainium2 Bass kernel for masked attention-pooling (DmasifAttentionModule).

Reference computation (per sample b):
    proj   = x @ W.T + b                  # [N, D]
    scores = proj @ v                     # [N]
    scores = where(mask, scores, -1e9)
    w      = softmax(scores)              # [N]
    out    = w @ x                        # [D]

Optimizations (exact up to fp reassociation unless noted):
  1. scores = x @ (W.T @ v) + (b . v); softmax is shift-invariant, so the
     (b . v) constant drops out and the 34-GFLOP projection collapses to a
     matvec against u = v @ W (host-computed, 512 floats).
  2. Masked rows get softmax weight exactly 0, so only the ~50% valid rows
     participate. The host compacts each sample to its valid rows (padded
     to a common column count with zero rows) and streams only those.
  3. x and u ship as fp16: halves HBM traffic (the binding resource), runs
     the pooling matmul at full PE rate (fp32 = 4 passes) and keeps DVE
     elementwise ops in 2x_1p mode. Score accumulation stays fp32.
  4. The compacted shard is host-swizzled to partition-major
     [128, SPB, ncols, D] so the whole 4.45 MiB arrives as ONE dma_start
     with 17.4 KiB contiguous per partition (meas. 295 GB/s vs 245 for
     per-tile strided transfers). Double-buffered across For_i iterations.
  5. Scores: a DVE free-dim reduce only has a 1x uop (694 ns/[128,512]
     column) while plain tensor_tensor runs 2x (438 ns), so columns are
     split: ~half fused on DVE (scalar_tensor_tensor w/ accum), the rest
     as DVE 2x products + ScalarE Copy-with-accum reduce (872 ns, ScalarE
     is otherwise idle). Masking is a single posthoc [128,ncols]
     tensor_add of -3e8 per masked column (no fp16-range contortions).
  6. exp: one batched ScalarE activation per sample (bias = -C shift),
     fp16 out. No accum: Z is recovered on host from the e tensor itself
     (8.7 KiB DMA per core), so numerator and denominator use bit-identical
     weights.
  7. Pooling: TensorE matvec accumulation into PSUM [1,512] per sample
     (lhsT = e column [128,1] fp16, rhs = x column [128,512] fp16,
     216 ns each); ScalarE copies PSUM out, host divides by Z.

Per-core budgets at ncols=17 (8 cores, 2 samples each, data-parallel):
DMA ~15.1 us, DVE ~18.7 us, ACT ~19 us, PE ~8 us -> ~20 us/iter steady.
"""

import os
import sys

import numpy as np

for _p in ("/opt/trn_rl_repo", "/root/.axon_site/_ro/trn_rl_repo"):
    if os.path.isdir(_p) and _p not in sys.path:
        sys.path.append(_p)

import concourse.bacc as bacc
import concourse.tile as tile
from concourse import mybir
from concourse.bass_utils import run_bass_kernel_spmd

B, N, D = 16, 4096, 512
N_CORES = 8
SPB = B // N_CORES          # samples per core
C_SHIFT = 24.0              # constant exp-range shift (softmax-invariant)
MASKED_INIT = -3.0e8        # masked scores -> exp underflows to exactly 0
ACT_COLS = 8                # score columns per sample reduced on ScalarE

_F32 = mybir.dt.float32
_F16 = mybir.dt.float16
_CACHE = {}


def _act_col_set(ncols, k):
    """k columns of [0,ncols) routed to the ScalarE reduce, spread out."""
    order = list(range(0, ncols, 2)) + list(range(1, ncols, 2))
    return frozenset(order[:k])


def _build_program(ncols, loop_n=None, act_cols=None, mask_in_stt=None):
    """Program for samples compacted to `ncols` columns of 128 rows each.

    loop_n wraps the computation in a HW For_i loop (timing only).
    mask_in_stt is accepted for test.py compatibility and ignored."""
    if act_cols is None:
        act_cols = ACT_COLS
    acols = _act_col_set(ncols, min(act_cols, ncols))

    nc = bacc.Bacc("TRN2", target_bir_lowering=False, debug=False)
    x = nc.dram_tensor("x", [128, SPB, ncols, D], _F16,
                       kind="ExternalInput").ap()
    mb = nc.dram_tensor("mb", [SPB, 128, ncols], _F32,
                        kind="ExternalInput").ap()
    u = nc.dram_tensor("u", [128, D], _F16, kind="ExternalInput").ap()
    out = nc.dram_tensor("out", [SPB, D], _F32, kind="ExternalOutput").ap()
    eout = nc.dram_tensor("eout", [128, SPB, ncols], _F16,
                          kind="ExternalOutput").ap()

    with tile.TileContext(nc) as tc:
        with (
            tc.tile_pool(name="xp", bufs=2) as xp,
            tc.tile_pool(name="singles", bufs=1) as sg,
            tc.tile_pool(name="prod", bufs=8) as prp,
            tc.tile_pool(name="scratch", bufs=2) as scr,
            tc.tile_pool(name="smalls", bufs=2) as sm,
            tc.tile_pool(name="ps", bufs=2, space="PSUM") as psp,
        ):
            ones_sb = sg.tile([128, 1], _F32)
            nc.vector.memset(ones_sb[:], 1.0)
            shift_sb = sg.tile([128, 1], _F32)
            nc.vector.memset(shift_sb[:], -C_SHIFT)
            warm = sg.tile([128, 1], _F32)
            # Pull the exp table-set load (~2.7us) to t=0, under the DMAs.
            nc.scalar.activation(warm[:], ones_sb[:],
                                 mybir.ActivationFunctionType.Exp)

            u_sb = sg.tile([128, D], _F16)
            nc.sync.dma_start(out=u_sb[:], in_=u[:])
            mb_sb = sg.tile([128, SPB, ncols], _F32)
            nc.sync.dma_start(out=mb_sb[:], in_=mb.rearrange("s p c -> p s c"))

            ctx = (nc, xp, prp, scr, sm, psp, x, out, eout, u_sb, mb_sb,
                   shift_sb, ncols, acols)

            if loop_n is not None:
                # For_i is a HW loop over a STATIC body: tile-pool rotation
                # only happens across emit calls, so double buffering needs
                # the body unrolled x2 (iteration i+1's DMA lands in the
                # other buffer and overlaps iteration i's compute).
                assert loop_n % 2 == 0, loop_n
                with tc.For_i(0, loop_n // 2, 1) as _i:
                    _emit_iteration(*ctx)
                    _emit_iteration(*ctx)
            else:
                _emit_iteration(*ctx)

    nc.compile()
    return nc


def _emit_iteration(nc, xp, prp, scr, sm, psp, x, out, eout, u_sb, mb_sb,
                    shift_sb, ncols, acols):
    # One DMA for the whole shard; double-buffered across iterations so the
    # transfer of iteration i+1 overlaps the compute of iteration i.
    xt = xp.tile([128, SPB, ncols, D], _F16, name="xt")
    nc.sync.dma_start(out=xt[:], in_=x[:])
    # Per-emit score/e tiles (rotate with the unrolled halves) so the two
    # in-flight iterations never alias.
    s_sb = xp.tile([128, SPB, ncols], _F32, name="s_sb")
    e_sb = xp.tile([128, SPB, ncols], _F16, name="e_sb")

    pool_ps = {s: psp.tile([1, D], _F32, name=f"pool_ps_{s}")
               for s in range(SPB)}

    for s in range(SPB):
        # Scores. DVE 1x fused column: s[c] = sum_d x*u (694 ns). ACT-routed
        # column: DVE 2x product tile (438 ns) + ScalarE Copy-accum reduce
        # (872 ns). The split keeps both engines ~equally loaded.
        for c in range(ncols):
            if c in acols:
                prod = prp.tile([128, D], _F16, name="prod")
                nc.vector.tensor_tensor(
                    out=prod[:], in0=xt[:, s, c, :], in1=u_sb[:],
                    op=mybir.AluOpType.mult)
                dump32 = scr.tile([128, D], _F32, name="dump32")
                nc.scalar.activation(
                    dump32[:], prod[:],
                    mybir.ActivationFunctionType.Copy,
                    accum_out=s_sb[:, s, c:c + 1])
            else:
                dump = scr.tile([128, D], _F16, name="dump")
                nc.vector.scalar_tensor_tensor(
                    out=dump[:], in0=xt[:, s, c, :],
                    scalar=0.0, in1=u_sb[:],
                    op0=mybir.AluOpType.add, op1=mybir.AluOpType.mult,
                    accum_out=s_sb[:, s, c:c + 1])
        # Masking: valid rows +0, padding rows -3e8 -> exp == 0 exactly.
        nc.vector.tensor_add(s_sb[:, s, :], s_sb[:, s, :], mb_sb[:, s, :])
        # e = exp(s - C), one batched op per sample, fp16 out for the PE.
        nc.scalar.activation(e_sb[:, s, :], s_sb[:, s, :],
                             mybir.ActivationFunctionType.Exp,
                             bias=shift_sb[:])
        # Pooling: accumulate e_c . x_c into PSUM [1, D].
        for c in range(ncols):
            nc.tensor.matmul(
                pool_ps[s][:],
                e_sb[:, s, c:c + 1],
                xt[:, s, c, :],
                start=(c == 0),
                stop=(c == ncols - 1),
            )
        # Ship the raw PSUM accumulator; host does out = raw/Z with Z from e.
        o_sb = sm.tile([1, D], _F32, name=f"o_{s}")
        nc.scalar.activation(o_sb[:], pool_ps[s][:],
                             mybir.ActivationFunctionType.Copy)
        nc.sync.dma_start(out=out[s:s + 1, :], in_=o_sb[:])
    nc.sync.dma_start(out=eout[:], in_=e_sb[:])


def _get_program(ncols):
    if ncols not in _CACHE:
        _CACHE[ncols] = _build_program(ncols)
    return _CACHE[ncols]


def _prep_inputs(x, flat_mask, W, v):
    """Compact to valid rows, swizzle partition-major; (in_maps, meta)."""
    x = np.ascontiguousarray(x, dtype=np.float32)
    flat_mask = np.asarray(flat_mask)
    W = np.asarray(W, dtype=np.float32)
    v = np.asarray(v, dtype=np.float32)
    # scores = x @ u + (b . v); the constant is dropped by softmax invariance.
    u = (v @ W).astype(np.float16)
    u_rep = np.ascontiguousarray(np.broadcast_to(u, (128, D)), dtype=np.float16)

    idxs = [np.nonzero(flat_mask[b] == 1)[0] for b in range(B)]
    counts = np.array([len(ix) for ix in idxs])
    ncols = max(1, int(-(-counts.max() // 128)))
    ncap = ncols * 128

    xc = np.zeros((B, ncap, D), dtype=np.float16)
    mbc = np.full((B, ncap), np.float32(MASKED_INIT), dtype=np.float32)
    for b in range(B):
        cnt = counts[b]
        if cnt:
            xc[b, :cnt] = x[b, idxs[b]]
            mbc[b, :cnt] = 0.0
    # row = col*128 + p  ->  [B, 128, ncols(, D)] partition-major
    xc = xc.reshape(B, ncols, 128, D).transpose(0, 2, 1, 3)
    mbc = np.ascontiguousarray(mbc.reshape(B, ncols, 128).transpose(0, 2, 1))

    in_maps = []
    for core in range(N_CORES):
        lo = core * SPB
        in_maps.append({
            # [128, SPB, ncols, D]
            "x": np.ascontiguousarray(xc[lo:lo + SPB].transpose(1, 0, 2, 3)),
            "mb": np.ascontiguousarray(mbc[lo:lo + SPB]),
            "u": u_rep,
        })
    meta = {"ncols": ncols, "mask_in_stt": False, "counts": counts}
    return in_maps, meta


def kernel(x, flat_mask, W, b, v, **_unused):
    in_maps, meta = _prep_inputs(x, flat_mask, W, v)
    nc = _get_program(meta["ncols"])
    res = run_bass_kernel_spmd(nc, in_maps, core_ids=list(range(N_CORES)))
    raw = np.concatenate([res.results[i]["out"] for i in range(N_CORES)],
                         axis=0)
    z = np.concatenate(
        [res.results[i]["eout"].astype(np.float32).sum(axis=(0, 2))
         for i in range(N_CORES)], axis=0)
    out = (raw / z[:, None]).astype(np.float32)
    if (meta["counts"] == 0).any():
        # Reference semantics for an all-masked sample: uniform mean pool.
        x = np.asarray(x, dtype=np.float32)
        for bi in np.nonzero(meta["counts"] == 0)[0]:
            out[bi] = x[bi].mean(axis=0)
    return out
